# revision 31
# baseline (speedup 1.0000x reference)
"""Trainium2 Bass kernel: multi-head self-attention with RoPE + sigmoid gating.

Computes, for fixed shapes B=2, S=2048, E=1024, H=16, D=64:
    qkv = x @ w_qkv ; q,k roped (concatenated-halves layout)
    att = softmax(q k^T / sqrt(D)) ; out = (att @ v * sigmoid(x @ w_gate)) @ w_out + b_out

Sharding: 8 cores = 2 (batch) x 4 (head groups of 4 heads).  Each core computes a
row-parallel partial of the output projection for its batch (its 4 heads' slice of
the E contraction); the host sums the 4 partials per batch and adds b_out.

All matmuls run in fp16 (inputs pre-cast host-side); PSUM accumulation is fp32.
fp16 (e5m10) costs the same as bf16 on every engine but carries 8x less
quantization noise; scores*scale stay in [-7.8, 7.8] so exp() fits fp16 range
with 3x margin and the softmax skips max-subtraction.  The softmax denominator
rides along as a 65th "ones" column of the PV matmul's stationary operand.

Schedule: K proj -> V proj -> Q(first half) proj, then 4 attention chunks of
512 q positions.  The attention phase is Scalar-engine bound (exp), so the
remaining matmul work (Q second half, gate projection, output projection of the
previous chunk) is interleaved into the chunks' PE streams as "filler" units to
keep the PE dense and fully ramped.

Device-side layouts (per core; host preps/permutes/casts all of these):
    xT   [1024, 2048]  x[b]^T
    wqk  [1024, 512]   columns: [q_even | q_odd | k_even | k_odd], each 128 = 4 heads x 32
    wv   [1024, 256]   v columns for the 4 heads (natural order)
    wg   [1024, 256]   w_gate columns for the 4 heads' output dims
    wo   [128, 2, 1024] w_out rows for the 4 heads, as 2 pair-tiles of 128
    cs/sn [128, 2048]  cos/sin RoPE tables, rows = 4x32 freqs, cols = position
Output: out [2048, 1024] fp16 partial (no bias); host sums in fp32.
"""

import numpy as np

B, S, E, H, D = 2, 2048, 1024, 16, 64
HC = 4            # heads per core
NCORES = 8
KT = E // 128     # 8 contraction tiles
ST = S // 128     # 16 sequence tiles
SQ = 512          # attention sq chunk
NCH = S // SQ     # 4 chunks
ROPE_THETA = 10000.0

_CACHE = {}

# Results of the most recent kernel() call, for test harnesses.
LAST_RESULTS = None


# ---------------------------------------------------------------------------
# BIR postprocess: the walrus build in this image accepts only ONE sync-wait
# command per lowered TPB instruction (Drain/NoOp/LDWEIGHTS/...).  Tile emits
# instructions with several waits; split the excess onto preceding single-wait
# NoOps on the same engine (program order preserves the blocking semantics).
# Installed by patching concourse's compile_bir_kernel in this process.
# ---------------------------------------------------------------------------

def _split_waits(bir_bytes, limit=1):
    import json as _json
    m = _json.loads(bir_bytes)
    counter = [0]

    def fix_block(instrs):
        out = []
        for ins in instrs:
            w = ins.get("sync_info", {}).get("on_wait", [])
            if len(w) > limit:
                chunks = [w[i:i + limit] for i in range(0, len(w), limit)]
                ins["sync_info"]["on_wait"] = chunks[-1]
                for ch in chunks[:-1]:
                    counter[0] += 1
                    out.append({
                        "name": f"I-waitsplit-{counter[0]}",
                        "opcode": "NoOp",
                        "engine": ins.get("engine"),
                        "ins": [],
                        "outs": [],
                        "sync_info": {"on_update": [], "on_wait": ch},
                    })
            out.append(ins)
        return out

    def walk(d):
        if isinstance(d, dict):
            for k, v in d.items():
                if k == "instructions" and isinstance(v, list):
                    d[k] = fix_block(v)
                else:
                    walk(v)
        elif isinstance(d, list):
            for v in d:
                walk(v)

    walk(m)
    return _json.dumps(m).encode()


def _install_birfix():
    if _CACHE.get("birfix"):
        return
    _CACHE["birfix"] = True
    import concourse.bass_utils as bu
    import concourse.bass2jax as b2j

    orig = bu.compile_bir_kernel

    def patched(bir_json, tmpdir, neff_name="file.neff"):
        return orig(_split_waits(bir_json), tmpdir, neff_name=neff_name)

    bu.compile_bir_kernel = patched
    b2j.compile_bir_kernel = patched


def _build_nc():
    import concourse.bass as bass
    import concourse.mybir as mybir
    from concourse.tile import TileContext

    f16 = mybir.dt.float16
    f32 = mybir.dt.float32
    MUL = mybir.AluOpType.mult
    SUB = mybir.AluOpType.subtract
    ADD = mybir.AluOpType.add
    Act = mybir.ActivationFunctionType

    nc = bass.Bass()
    xT_d = nc.dram_tensor("xT", (E, S), f16, kind="ExternalInput")
    wqk_d = nc.dram_tensor("wqk", (E, 4 * 128), f16, kind="ExternalInput")
    wv_d = nc.dram_tensor("wv", (E, HC * 64), f16, kind="ExternalInput")
    wg_d = nc.dram_tensor("wg", (E, HC * 64), f16, kind="ExternalInput")
    wo_d = nc.dram_tensor("wo", (128, 2, E), f16, kind="ExternalInput")
    cs_d = nc.dram_tensor("cs", (128, S), f16, kind="ExternalInput")
    sn_d = nc.dram_tensor("sn", (128, S), f16, kind="ExternalInput")
    out_d = nc.dram_tensor("out", (S, E), f16, kind="ExternalOutput")

    with TileContext(nc) as tc:
        with (
            tc.tile_pool(name="const", bufs=1) as cpool,
            tc.tile_pool(name="big", bufs=1) as bpool,
            tc.tile_pool(name="work", bufs=3) as wpool,
            tc.tile_pool(name="expool", bufs=8) as expool,
            tc.tile_pool(name="outp", bufs=3) as opool,
            tc.tile_pool(name="scp", bufs=2, space="PSUM") as scpool,
            tc.tile_pool(name="pvp", bufs=1, space="PSUM") as pvpool,
        ):
            # ---- constants / weights in, ordered by first use: the K-half of
            # wqk + xT half 0 gate the first PE matmuls ----
            wqk = cpool.tile([128, KT, 512], f16)
            nc.sync.dma_start(wqk[:, :, 256:512],
                              wqk_d[:, 256:512].rearrange("(k p) m -> p k m", p=128))
            xT = bpool.tile([128, KT, S], f16)
            for half in range(2):
                sl = slice(half * 1024, (half + 1) * 1024)
                for k in range(KT):
                    nc.sync.dma_start(xT[:, k, sl], xT_d[k * 128:(k + 1) * 128, sl])
                if half == 0:
                    nc.sync.dma_start(wqk[:, :, 0:256],
                                      wqk_d[:, 0:256].rearrange("(k p) m -> p k m", p=128))
                    cs = cpool.tile([128, S], f16)
                    nc.sync.dma_start(cs, cs_d[:, :])
                    sn = cpool.tile([128, S], f16)
                    nc.sync.dma_start(sn, sn_d[:, :])
            wv = cpool.tile([128, KT, 256], f16)
            nc.sync.dma_start(wv, wv_d[:, :].rearrange("(k p) m -> p k m", p=128))
            wg = cpool.tile([128, KT, 256], f16)
            nc.sync.dma_start(wg, wg_d[:, :].rearrange("(k p) m -> p k m", p=128))
            wo = cpool.tile([128, 2, E], f16)
            nc.sync.dma_start(wo, wo_d[:, :, :])

            # warm the ACT exp/tanh table set before it is on the critical path
            warm = cpool.tile([1, 8], f32)
            nc.vector.memset(warm, 0.0)
            nc.scalar.activation(warm, warm, Act.Exp)

            # one-hot rows for the matmul-based partition broadcast of the
            # softmax denominators: hot[32h, 128h + r] = 1
            hot = cpool.tile([128, HC * 128], f16)
            nc.vector.memset(hot, 0.0)
            for h in range(HC):
                nc.vector.memset(hot[32 * h:32 * h + 1, 128 * h:128 * (h + 1)], 1.0)

            # ---------------- projection helpers ----------------
            # m-tile indices into wqk: 0=q_even 1=q_odd 2=k_even 3=k_odd
            def proj_qk_unit(dst, m, n2, halves=(0, 1)):
                """Qk-projection psum accumulation over (seq slice n2, halves).
                As an attention-phase filler, pass a single half: a 512-col
                unit holds the score ring ~1.7 us instead of ~3.4, which
                halves the bubble it pokes into the exp stream."""
                for half in halves:
                    o = 512 * half
                    ps = scpool.tile([128, 512], f32, tag="sc", name="ps_qk")
                    for k in range(KT):
                        nc.tensor.matmul(
                            ps,
                            lhsT=wqk[:, k, m * 128:(m + 1) * 128],
                            rhs=xT[:, k, n2 * 1024 + o:n2 * 1024 + o + 512],
                            start=(k == 0), stop=(k == KT - 1),
                        )
                    nc.vector.tensor_copy(dst[:, n2 * 1024 + o:n2 * 1024 + o + 512], ps)

            def rope_unit(ev, od, top, bot, n2):
                """RoPE for one 1024-column seq slice (full 128-partition ops)."""
                sl = slice(n2 * 1024, (n2 + 1) * 1024)
                t1 = wpool.tile([128, 1024], f16, tag="rt1", name="rt1")
                t2 = wpool.tile([128, 1024], f16, tag="rt2", name="rt2")
                nc.vector.tensor_tensor(t1, ev[:, sl], cs[:, sl], MUL)
                nc.vector.tensor_tensor(t2, od[:, sl], sn[:, sl], MUL)
                nc.vector.tensor_tensor(top[:, sl], t1, t2, SUB)
                t3 = wpool.tile([128, 1024], f16, tag="rt1", name="rt3")
                t4 = wpool.tile([128, 1024], f16, tag="rt2", name="rt4")
                nc.vector.tensor_tensor(t3, ev[:, sl], sn[:, sl], MUL)
                nc.vector.tensor_tensor(t4, od[:, sl], cs[:, sl], MUL)
                nc.vector.tensor_tensor(bot[:, sl], t3, t4, ADD)

            def assemble_unit(top, bot, dst, n2):
                """Pack roped halves into pair-tile rows for 2 heads (one g)."""
                sl = slice(n2 * 1024, (n2 + 1) * 1024)
                g = dst_g[id(dst)]
                for h2 in range(2):
                    h = 2 * g + h2
                    nc.sync.dma_start(dst[64 * h2:64 * h2 + 32, sl], top[32 * h:32 * h + 32, sl])
                    nc.sync.dma_start(dst[64 * h2 + 32:64 * h2 + 64, sl], bot[32 * h:32 * h + 32, sl])

            def gate_unit(g, n2, halves=(0, 1)):
                """Gate projection -> sigmoid for (g, seq slice n2, halves)."""
                for half in halves:
                    o = n2 * 1024 + 512 * half
                    ps = scpool.tile([128, 512], f32, tag="sc", name="ps_g")
                    for k in range(KT):
                        nc.tensor.matmul(
                            ps,
                            lhsT=wg[:, k, g * 128:(g + 1) * 128],
                            rhs=xT[:, k, o:o + 512],
                            start=(k == 0), stop=(k == KT - 1),
                        )
                    th = wpool.tile([128, 512], f16, tag="th", name="th")
                    nc.scalar.activation(th, ps, Act.Tanh, scale=0.5)
                    # sigmoid(x) = 0.5*tanh(x/2) + 0.5
                    nc.vector.tensor_scalar(gP[g][:, o:o + 512], th, 0.5, 0.5, MUL, ADD)

            def outproj_unit(c, st, ob_act=False):
                """Output projection for one 128-row s-tile of chunk c.
                ob_act routes the PSUM->SBUF unload through the Scalar engine
                (for chunk boundaries, where ACT is otherwise idle and the DVE
                is busy with the normalization chain)."""
                s = (SQ // 128) * c + st
                ps = scpool.tile([128, 1024], f32, tag="sc", name="ps_o")
                for n in range(2):
                    for g in range(2):
                        nc.tensor.matmul(
                            ps[:, n * 512:(n + 1) * 512],
                            lhsT=ag[g][:, s * 128:(s + 1) * 128],
                            rhs=wo[:, g, n * 512:(n + 1) * 512],
                            start=(g == 0), stop=(g == 1),
                        )
                ob = opool.tile([128, 1024], f16, tag="ob", name="ob")
                if ob_act:
                    nc.scalar.copy(ob, ps)
                else:
                    nc.vector.tensor_copy(ob, ps)
                nc.sync.dma_start(out_d[s * 128:(s + 1) * 128, :], ob)

            # ---------------- phase A: K, V, Q(first half) ----------------
            qTop = bpool.tile([128, S], f16)
            qBot = bpool.tile([128, S], f16)
            kTop = bpool.tile([128, S], f16)
            kBot = bpool.tile([128, S], f16)
            kraw = [bpool.tile([128, S], f16, tag=f"kraw{i}", name=f"kraw{i}") for i in range(2)]
            qraw = [bpool.tile([128, S], f16, tag=f"qraw{i}", name=f"qraw{i}") for i in range(2)]
            qR = [bpool.tile([128, S], f16, tag=f"qR{g}", name=f"qR{g}") for g in range(2)]
            kR = [bpool.tile([128, S], f16, tag=f"kR{g}", name=f"kR{g}") for g in range(2)]
            dst_g = {}
            for g in range(2):
                dst_g[id(qR[g])] = g
                dst_g[id(kR[g])] = g

            # K projection + rope + assembly (both halves -- attention needs all
            # kpos), then Q first half so its rope/assembly overlaps V proj.
            for n2 in range(2):
                proj_qk_unit(kraw[0], 2, n2)
                proj_qk_unit(kraw[1], 3, n2)
                rope_unit(kraw[0], kraw[1], kTop, kBot, n2)
                for g in range(2):
                    assemble_unit(kTop, kBot, kR[g], n2)

            # Q projection, first half (chunks 0/1) + rope + assembly
            proj_qk_unit(qraw[0], 0, 0)
            proj_qk_unit(qraw[1], 1, 0)
            rope_unit(qraw[0], qraw[1], qTop, qBot, 0)
            for g in range(2):
                assemble_unit(qTop, qBot, qR[g], 0)

            # V projection into [v_h | 1] stationary tiles
            vOnes = []
            for s in range(ST):
                vo = bpool.tile([128, HC * 65], f16, tag=f"vo{s}", name=f"vo{s}")
                vOnes.append(vo)
                nc.gpsimd.memset(vo, 1.0)
                ps = scpool.tile([128, 1024], f32, tag="sc", name="ps_v")
                for k in range(KT):
                    nc.tensor.matmul(
                        ps[:, :256],
                        lhsT=xT[:, k, s * 128:(s + 1) * 128],
                        rhs=wv[:, k, :],
                        start=(k == 0), stop=(k == KT - 1),
                    )
                nc.vector.tensor_copy(
                    vo.rearrange("p (h w) -> p h w", w=65)[:, :, 0:64],
                    ps[:, :256].rearrange("p (h w) -> p h w", w=64),
                )

            gP = [bpool.tile([128, S], f16, tag=f"gP{g}", name=f"gP{g}") for g in range(2)]
            ag = [bpool.tile([128, S], f16, tag=f"ag{g}", name=f"ag{g}") for g in range(2)]

            # Filler units to interleave into the (ACT-bound) attention chunks.
            # The LAST TWO of each list run at the chunk boundary to keep the
            # PE busy while the normalization chain drains on the DVE; their
            # PSUM unloads route through the idle Scalar engine (ob_act).
            # Constraints: gate(.,0) before norm(c0); gate(.,1) before norm(c1);
            # q second half + its rope before chunk 2's scores.
            def half_unit(fn, *args):
                return [lambda h=h: fn(*args, halves=(h,)) for h in range(2)]

            fillers = {
                0: [*half_unit(gate_unit, 0, 0), *half_unit(gate_unit, 1, 0),
                    *half_unit(proj_qk_unit, qraw[0], 0, 1),
                    *half_unit(proj_qk_unit, qraw[1], 1, 1)],
                1: [*half_unit(gate_unit, 0, 1), *half_unit(gate_unit, 1, 1),
                    lambda: outproj_unit(0, 0), lambda: outproj_unit(0, 1),
                    lambda: outproj_unit(0, 2, True), lambda: outproj_unit(0, 3, True)],
                2: [lambda: outproj_unit(1, 0), lambda: outproj_unit(1, 1),
                    lambda: outproj_unit(1, 2, True), lambda: outproj_unit(1, 3, True)],
                3: [lambda: outproj_unit(2, 0), lambda: outproj_unit(2, 1),
                    lambda: outproj_unit(2, 2, True), lambda: outproj_unit(2, 3, True)],
            }
            # extra non-PE work appended to the deferred norm tail of chunk 0:
            # rope + assembly for the second q half (needed by chunk 2)
            norm_extra = {
                0: lambda: (rope_unit(qraw[0], qraw[1], qTop, qBot, 1),
                            assemble_unit(qTop, qBot, qR[0], 1),
                            assemble_unit(qTop, qBot, qR[1], 1)),
            }

            # ---------------- attention chunks ----------------
            # PV matmuls lag the score/exp pipeline by one sk: they read ex
            # from the 6-deep SBUF ring, so the PE never sits directly behind
            # the Scalar engine in program order and both stream freely.
            scale = float(D) ** -0.5
            pending_norm = None
            carry = []   # boundary fillers, emitted in the NEXT chunk's sk 0/1
            for c in range(NCH):
                csl = slice(c * SQ, (c + 1) * SQ)
                # all 4 heads' PV accumulators in one 4-bank tile; row 64 of
                # bank h collects the softmax denominator via the ones column
                pv4 = pvpool.tile([128, HC, SQ], f32, tag="pv4", name="pv4")
                # memset early (off the boundary critical path): unwritten rows
                # must not hold NaN bit patterns (0 * NaN = NaN in the one-hot
                # broadcast matmul below)
                dstack = wpool.tile([128, SQ], f32, tag="dstack", name="dstack", bufs=2)
                nc.vector.memset(dstack, 1.0)

                def emit_pv(exs, sk):
                    for g in range(2):
                        for h2 in range(2):
                            h = 2 * g + h2
                            nc.tensor.matmul(
                                pv4[0:65, h, :],
                                lhsT=vOnes[sk][:, h * 65:(h + 1) * 65],
                                rhs=exs[g][:, h2 * 512:(h2 + 1) * 512],
                                start=(sk == 0), stop=(sk == ST - 1),
                            )

                todo = list(fillers[c])
                pend = []
                for sk in range(ST):
                    # PV lags TWO sk and is emitted BEFORE the scores: its ex
                    # semaphores are long posted, so it runs immediately and
                    # hides the score-ring slot-grant latency (keeping the PE
                    # dense and its DVFS ramp alive)
                    if len(pend) == 2:
                        emit_pv(*pend.pop(0))
                    exs = []
                    for g in range(2):
                        sct = scpool.tile([128, 1024], f32, tag="sc", name="sct")
                        for h2 in range(2):
                            nc.tensor.matmul(
                                sct[:, h2 * 512:(h2 + 1) * 512],
                                lhsT=kR[g][64 * h2:64 * (h2 + 1), sk * 128:(sk + 1) * 128],
                                rhs=qR[g][64 * h2:64 * (h2 + 1), csl],
                                start=True, stop=True,
                            )
                        ex = expool.tile([128, 1024], f16, tag="ex", name="ex")
                        nc.scalar.activation(ex, sct, Act.Exp, scale=scale)
                        exs.append(ex)
                    # previous chunk's boundary fillers: emitted between this
                    # chunk's first scores and its first PV, so the scores/exp
                    # pipeline restarts instantly at the boundary while the
                    # fillers cover the pv4-release wait
                    if sk < 2 and carry:
                        carry.pop(0)()
                    pend.append((exs, sk))
                    # deferred norm tail of the previous chunk: at sk 3 its
                    # recB tiles slot into the score ring without blocking the
                    # next scores (the reciprocal has drained by then)
                    if sk == 3 and pending_norm is not None:
                        pending_norm()
                        pending_norm = None
                    # spread fillers through the chunk, keeping two for the
                    # chunk boundary; start at sk 5, after the deferred norm
                    # tail (which writes ag) has landed
                    if sk % 2 == 1 and sk >= 5 and len(todo) > 2:
                        todo.pop(0)()
                for p in pend:
                    emit_pv(*p)

                if c == NCH - 1:
                    carry = todo
                    break  # handled by the pipelined tail below

                # ---- normalization, part 1 (DVE, drains under the boundary
                # fillers): denominators stacked on rows {0,32,64,96} so the
                # reciprocal runs partition-parallel (DVE recip costs ~6.3 ns
                # per FREE element), and the gate multiply unloads pv4.
                for h in range(HC):
                    nc.vector.tensor_copy(dstack[32 * h:32 * h + 1, :], pv4[64:65, h, :])
                uus = []
                for h in range(HC):
                    g, h2 = divmod(h, 2)
                    o = 64 * h2
                    uu = wpool.tile([128, SQ], f16, tag=f"uu{h}", name=f"uu{h}", bufs=1)
                    nc.vector.tensor_tensor(uu[o:o + 64, :], pv4[0:64, h, :],
                                            gP[g][o:o + 64, csl], MUL)
                    uus.append(uu)
                rec128 = wpool.tile([128, SQ], f16, tag="rec128", name="rec128", bufs=2)
                with nc.allow_low_precision(reason="fp16 denom recip: denom in [2.4e3, 9e3], rel err 2^-11"):
                    nc.vector.reciprocal(out=rec128, in_=dstack)

                # leftover fillers become the next chunk's boundary cover
                carry = todo

                # ---- normalization, part 2 (deferred into the next chunk's
                # sk loop): one-hot fp16 matmuls broadcast each head's
                # reciprocal row across partitions, then ag = uu * recB.
                def make_norm_tail(c=c, csl=csl, rec128=rec128, uus=uus):
                    def fin():
                        for g in range(2):
                            recB2 = scpool.tile([128, 1024], f32, tag="sc", name="recB2")
                            for h2 in range(2):
                                h = 2 * g + h2
                                nc.tensor.matmul(recB2[:, h2 * 512:(h2 + 1) * 512],
                                                 lhsT=hot[:, 128 * h:128 * (h + 1)],
                                                 rhs=rec128, start=True, stop=True)
                            for h2 in range(2):
                                h = 2 * g + h2
                                o = 64 * h2
                                nc.vector.tensor_tensor(
                                    ag[g][o:o + 64, csl],
                                    uus[h][o:o + 64, :],
                                    recB2[o:o + 64, h2 * 512:(h2 + 1) * 512],
                                    MUL,
                                )
                        extra = norm_extra.get(c)
                        if extra:
                            extra()
                    return fin
                pending_norm = make_norm_tail()

            # ---- pipelined tail: the last chunk's normalization runs in two
            # 256-column halves so the output projection of half A overlaps
            # half B's reciprocal chain; all PSUM unloads go through ACT.
            c3 = NCH - 1
            csl3 = slice(c3 * SQ, (c3 + 1) * SQ)
            rec128 = wpool.tile([128, SQ], f16, tag="rec128", name="rec128", bufs=2)
            uus = [wpool.tile([128, SQ], f16, tag=f"uu{h}", name=f"uu{h}", bufs=1)
                   for h in range(HC)]
            first = True
            for qo in (0, 256):
                qsl = slice(qo, qo + 256)
                for h in range(HC):
                    nc.vector.tensor_copy(dstack[32 * h:32 * h + 1, qsl],
                                          pv4[64:65, h, qsl])
                for h in range(HC):
                    g, h2 = divmod(h, 2)
                    o = 64 * h2
                    nc.vector.tensor_tensor(
                        uus[h][o:o + 64, qsl], pv4[0:64, h, qsl],
                        gP[g][o:o + 64, c3 * SQ + qo:c3 * SQ + qo + 256], MUL)
                with nc.allow_low_precision(reason="fp16 denom recip"):
                    nc.vector.reciprocal(out=rec128[:, qsl], in_=dstack[:, qsl])
                if first:
                    for f in carry:  # PE cover for half A's chain
                        f()
                    first = False
                for g in range(2):
                    recB2 = scpool.tile([128, 1024], f32, tag="sc", name="recB2")
                    for h2 in range(2):
                        h = 2 * g + h2
                        nc.tensor.matmul(recB2[:, h2 * 256:(h2 + 1) * 256],
                                         lhsT=hot[:, 128 * h:128 * (h + 1)],
                                         rhs=rec128[:, qsl], start=True, stop=True)
                    for h2 in range(2):
                        h = 2 * g + h2
                        o = 64 * h2
                        nc.vector.tensor_tensor(
                            ag[g][o:o + 64, c3 * SQ + qo:c3 * SQ + qo + 256],
                            uus[h][o:o + 64, qsl],
                            recB2[o:o + 64, h2 * 256:(h2 + 1) * 256],
                            MUL,
                        )
                for st in (0, 1) if qo == 0 else (2, 3):
                    outproj_unit(c3, st, ob_act=True)

    return nc


def _host_inputs(x, w_qkv, w_gate, w_out):
    """Build the 8 per-core input maps (all device tensors fp16)."""
    f16 = np.float16
    x = np.asarray(x, dtype=np.float32)
    w_qkv = np.asarray(w_qkv, dtype=np.float32)
    w_gate = np.asarray(w_gate, dtype=np.float32)
    w_out = np.asarray(w_out, dtype=np.float32)

    inv = 1.0 / (ROPE_THETA ** (np.arange(0, D, 2, dtype=np.float64) / D))   # [32]
    ang = np.arange(S, dtype=np.float64)[None, :] * inv[:, None]             # [32, S]
    cs = np.tile(np.cos(ang), (4, 1)).astype(f16)                            # [128, S]
    sn = np.tile(np.sin(ang), (4, 1)).astype(f16)

    wq = w_qkv[:, 0:E]
    wk = w_qkv[:, E:2 * E]
    wvv = w_qkv[:, 2 * E:3 * E]

    in_maps = []
    for c in range(NCORES):
        b = c // 4
        hs = HC * (c % 4)
        cols_ev = np.concatenate([(hs + h) * 64 + np.arange(0, 64, 2) for h in range(HC)])
        cols_od = cols_ev + 1
        wqk_p = np.concatenate(
            [wq[:, cols_ev], wq[:, cols_od], wk[:, cols_ev], wk[:, cols_od]], axis=1)
        vcols = np.concatenate([(hs + h) * 64 + np.arange(64) for h in range(HC)])
        wo_p = w_out[vcols, :].reshape(2, 128, E).transpose(1, 0, 2)
        in_maps.append({
            "xT": np.ascontiguousarray(x[b].T).astype(f16),
            "wqk": np.ascontiguousarray(wqk_p).astype(f16),
            "wv": np.ascontiguousarray(wvv[:, vcols]).astype(f16),
            "wg": np.ascontiguousarray(w_gate[:, vcols]).astype(f16),
            "wo": np.ascontiguousarray(wo_p).astype(f16),
            "cs": cs,
            "sn": sn,
        })
    return in_maps


def kernel(x, w_qkv, w_gate, w_out, b_out, n_heads):
    global LAST_RESULTS
    assert int(n_heads) == H
    x = np.asarray(x)
    assert x.shape == (B, S, E)

    from concourse.bass_utils import run_bass_kernel_spmd

    _install_birfix()
    if "nc" not in _CACHE:
        _CACHE["nc"] = _build_nc()
    nc = _CACHE["nc"]

    in_maps = _host_inputs(x, w_qkv, w_gate, w_out)
    import os
    trace = bool(int(os.environ.get("KERNEL_TRACE", "0")))
    tmpdir = os.environ.get("KERNEL_TRACE_DIR") if trace else None
    res = run_bass_kernel_spmd(nc, in_maps, list(range(NCORES)), trace=trace,
                               tmpdir=tmpdir)
    LAST_RESULTS = res

    out = np.zeros((B, S, E), dtype=np.float32)
    for c in range(NCORES):
        out[c // 4] += res.results[c]["out"].astype(np.float32)
    out += np.asarray(b_out, dtype=np.float32)[None, None, :]
    return out


# revision 33
# speedup vs baseline: 1.0336x; 1.0336x over previous
"""Trainium2 Bass kernel: multi-head self-attention with RoPE + sigmoid gating.

Computes, for fixed shapes B=2, S=2048, E=1024, H=16, D=64:
    qkv = x @ w_qkv ; q,k roped (concatenated-halves layout)
    att = softmax(q k^T / sqrt(D)) ; out = (att @ v * sigmoid(x @ w_gate)) @ w_out + b_out

Sharding: 8 cores = 2 (batch) x 4 (head groups of 4 heads).  Each core computes a
row-parallel partial of the output projection for its batch (its 4 heads' slice of
the E contraction); the host sums the 4 partials per batch and adds b_out.

All matmuls run in fp16 (inputs pre-cast host-side); PSUM accumulation is fp32.
fp16 (e5m10) costs the same as bf16 on every engine but carries 8x less
quantization noise; scores*scale stay in [-7.8, 7.8] so exp() fits fp16 range
with 3x margin and the softmax skips max-subtraction.  The softmax denominator
rides along as a 65th "ones" column of the PV matmul's stationary operand.

Schedule: K proj -> V proj -> Q(first half) proj, then 4 attention chunks of
512 q positions.  The attention phase is Scalar-engine bound (exp), so the
remaining matmul work (Q second half, gate projection, output projection of the
previous chunk) is interleaved into the chunks' PE streams as "filler" units to
keep the PE dense and fully ramped.

Device-side layouts (per core; host preps/permutes/casts all of these):
    xT   [1024, 2048]  x[b]^T
    wqk  [1024, 512]   columns: [q_even | q_odd | k_even | k_odd], each 128 = 4 heads x 32
    wv   [1024, 256]   v columns for the 4 heads (natural order)
    wg   [1024, 256]   w_gate columns for the 4 heads' output dims
    wo   [128, 2, 1024] w_out rows for the 4 heads, as 2 pair-tiles of 128
    cs/sn [128, 2048]  cos/sin RoPE tables, rows = 4x32 freqs, cols = position
Output: out [2048, 1024] fp16 partial (no bias); host sums in fp32.
"""

import numpy as np

B, S, E, H, D = 2, 2048, 1024, 16, 64
HC = 4            # heads per core
NCORES = 8
KT = E // 128     # 8 contraction tiles
ST = S // 128     # 16 sequence tiles
SQ = 512          # attention sq chunk
NCH = S // SQ     # 4 chunks
ROPE_THETA = 10000.0

_CACHE = {}

# Results of the most recent kernel() call, for test harnesses.
LAST_RESULTS = None


# ---------------------------------------------------------------------------
# BIR postprocess: the walrus build in this image accepts only ONE sync-wait
# command per lowered TPB instruction (Drain/NoOp/LDWEIGHTS/...).  Tile emits
# instructions with several waits; split the excess onto preceding single-wait
# NoOps on the same engine (program order preserves the blocking semantics).
# Installed by patching concourse's compile_bir_kernel in this process.
# ---------------------------------------------------------------------------

def _split_waits(bir_bytes, limit=1):
    import json as _json
    m = _json.loads(bir_bytes)
    counter = [0]

    def fix_block(instrs):
        out = []
        for ins in instrs:
            w = ins.get("sync_info", {}).get("on_wait", [])
            if len(w) > limit:
                chunks = [w[i:i + limit] for i in range(0, len(w), limit)]
                ins["sync_info"]["on_wait"] = chunks[-1]
                for ch in chunks[:-1]:
                    counter[0] += 1
                    out.append({
                        "name": f"I-waitsplit-{counter[0]}",
                        "opcode": "NoOp",
                        "engine": ins.get("engine"),
                        "ins": [],
                        "outs": [],
                        "sync_info": {"on_update": [], "on_wait": ch},
                    })
            out.append(ins)
        return out

    def walk(d):
        if isinstance(d, dict):
            for k, v in d.items():
                if k == "instructions" and isinstance(v, list):
                    d[k] = fix_block(v)
                else:
                    walk(v)
        elif isinstance(d, list):
            for v in d:
                walk(v)

    walk(m)
    return _json.dumps(m).encode()


def _install_birfix():
    if _CACHE.get("birfix"):
        return
    _CACHE["birfix"] = True
    import concourse.bass_utils as bu
    import concourse.bass2jax as b2j

    orig = bu.compile_bir_kernel

    def patched(bir_json, tmpdir, neff_name="file.neff"):
        return orig(_split_waits(bir_json), tmpdir, neff_name=neff_name)

    bu.compile_bir_kernel = patched
    b2j.compile_bir_kernel = patched


def _build_nc():
    import concourse.bass as bass
    import concourse.mybir as mybir
    from concourse.tile import TileContext

    f16 = mybir.dt.float16
    f32 = mybir.dt.float32
    MUL = mybir.AluOpType.mult
    SUB = mybir.AluOpType.subtract
    ADD = mybir.AluOpType.add
    Act = mybir.ActivationFunctionType

    nc = bass.Bass()
    xT_d = nc.dram_tensor("xT", (E, S), f16, kind="ExternalInput")
    wqk_d = nc.dram_tensor("wqk", (E, 4 * 128), f16, kind="ExternalInput")
    wv_d = nc.dram_tensor("wv", (E, HC * 64), f16, kind="ExternalInput")
    wg_d = nc.dram_tensor("wg", (E, HC * 64), f16, kind="ExternalInput")
    wo_d = nc.dram_tensor("wo", (128, 2, E), f16, kind="ExternalInput")
    cs_d = nc.dram_tensor("cs", (128, S), f16, kind="ExternalInput")
    sn_d = nc.dram_tensor("sn", (128, S), f16, kind="ExternalInput")
    out_d = nc.dram_tensor("out", (S, E), f16, kind="ExternalOutput")

    with TileContext(nc) as tc:
        with (
            tc.tile_pool(name="const", bufs=1) as cpool,
            tc.tile_pool(name="big", bufs=1) as bpool,
            tc.tile_pool(name="work", bufs=3) as wpool,
            tc.tile_pool(name="expool", bufs=8) as expool,
            tc.tile_pool(name="outp", bufs=3) as opool,
            tc.tile_pool(name="scp", bufs=2, space="PSUM") as scpool,
            tc.tile_pool(name="pvp", bufs=1, space="PSUM") as pvpool,
        ):
            # ---- constants / weights in, ordered by first use: the K-half of
            # wqk + xT half 0 gate the first PE matmuls ----
            wqk = cpool.tile([128, KT, 512], f16)
            nc.sync.dma_start(wqk[:, :, 256:512],
                              wqk_d[:, 256:512].rearrange("(k p) m -> p k m", p=128))
            xT = bpool.tile([128, KT, S], f16)
            for half in range(2):
                sl = slice(half * 1024, (half + 1) * 1024)
                for k in range(KT):
                    nc.sync.dma_start(xT[:, k, sl], xT_d[k * 128:(k + 1) * 128, sl])
                if half == 0:
                    nc.sync.dma_start(wqk[:, :, 0:256],
                                      wqk_d[:, 0:256].rearrange("(k p) m -> p k m", p=128))
                    cs = cpool.tile([128, S], f16)
                    nc.sync.dma_start(cs, cs_d[:, :])
                    sn = cpool.tile([128, S], f16)
                    nc.sync.dma_start(sn, sn_d[:, :])
            wv = cpool.tile([128, KT, 256], f16)
            nc.sync.dma_start(wv, wv_d[:, :].rearrange("(k p) m -> p k m", p=128))
            wg = cpool.tile([128, KT, 256], f16)
            nc.sync.dma_start(wg, wg_d[:, :].rearrange("(k p) m -> p k m", p=128))
            wo = cpool.tile([128, 2, E], f16)
            nc.sync.dma_start(wo, wo_d[:, :, :])

            # warm the ACT exp/tanh table set before it is on the critical path
            warm = cpool.tile([1, 8], f32)
            nc.vector.memset(warm, 0.0)
            nc.scalar.activation(warm, warm, Act.Exp)

            # one-hot rows for the matmul-based partition broadcast of the
            # softmax denominators: hot[32h, 128h + r] = 1
            hot = cpool.tile([128, HC * 128], f16)
            nc.vector.memset(hot, 0.0)
            for h in range(HC):
                nc.vector.memset(hot[32 * h:32 * h + 1, 128 * h:128 * (h + 1)], 1.0)

            # ---------------- projection helpers ----------------
            # m-tile indices into wqk: 0=q_even 1=q_odd 2=k_even 3=k_odd
            def proj_qk_unit(dst, m, n2, halves=(0, 1)):
                """Qk-projection psum accumulation over (seq slice n2, halves).
                As an attention-phase filler, pass a single half: a 512-col
                unit holds the score ring ~1.7 us instead of ~3.4, which
                halves the bubble it pokes into the exp stream."""
                for half in halves:
                    o = 512 * half
                    ps = scpool.tile([128, 512], f32, tag="sc", name="ps_qk")
                    for k in range(KT):
                        nc.tensor.matmul(
                            ps,
                            lhsT=wqk[:, k, m * 128:(m + 1) * 128],
                            rhs=xT[:, k, n2 * 1024 + o:n2 * 1024 + o + 512],
                            start=(k == 0), stop=(k == KT - 1),
                        )
                    nc.vector.tensor_copy(dst[:, n2 * 1024 + o:n2 * 1024 + o + 512], ps)

            def rope_unit(ev, od, top, bot, n2):
                """RoPE for one 1024-column seq slice (full 128-partition ops)."""
                sl = slice(n2 * 1024, (n2 + 1) * 1024)
                t1 = wpool.tile([128, 1024], f16, tag="rt1", name="rt1")
                t2 = wpool.tile([128, 1024], f16, tag="rt2", name="rt2")
                nc.vector.tensor_tensor(t1, ev[:, sl], cs[:, sl], MUL)
                nc.vector.tensor_tensor(t2, od[:, sl], sn[:, sl], MUL)
                nc.vector.tensor_tensor(top[:, sl], t1, t2, SUB)
                t3 = wpool.tile([128, 1024], f16, tag="rt1", name="rt3")
                t4 = wpool.tile([128, 1024], f16, tag="rt2", name="rt4")
                nc.vector.tensor_tensor(t3, ev[:, sl], sn[:, sl], MUL)
                nc.vector.tensor_tensor(t4, od[:, sl], cs[:, sl], MUL)
                nc.vector.tensor_tensor(bot[:, sl], t3, t4, ADD)

            def assemble_unit(top, bot, dst, n2):
                """Pack roped halves into pair-tile rows for 2 heads (one g)."""
                sl = slice(n2 * 1024, (n2 + 1) * 1024)
                g = dst_g[id(dst)]
                for h2 in range(2):
                    h = 2 * g + h2
                    nc.sync.dma_start(dst[64 * h2:64 * h2 + 32, sl], top[32 * h:32 * h + 32, sl])
                    nc.sync.dma_start(dst[64 * h2 + 32:64 * h2 + 64, sl], bot[32 * h:32 * h + 32, sl])

            def gate_unit(g, n2, halves=(0, 1)):
                """Gate projection -> sigmoid for (g, seq slice n2, halves)."""
                for half in halves:
                    o = n2 * 1024 + 512 * half
                    ps = scpool.tile([128, 512], f32, tag="sc", name="ps_g")
                    for k in range(KT):
                        nc.tensor.matmul(
                            ps,
                            lhsT=wg[:, k, g * 128:(g + 1) * 128],
                            rhs=xT[:, k, o:o + 512],
                            start=(k == 0), stop=(k == KT - 1),
                        )
                    th = wpool.tile([128, 512], f16, tag="th", name="th")
                    nc.scalar.activation(th, ps, Act.Tanh, scale=0.5)
                    # sigmoid(x) = 0.5*tanh(x/2) + 0.5
                    nc.vector.tensor_scalar(gP[g][:, o:o + 512], th, 0.5, 0.5, MUL, ADD)

            def outproj_unit(c, st, ob_act=False):
                """Output projection for one 128-row s-tile of chunk c.
                ob_act routes the PSUM->SBUF unload through the Scalar engine
                (for chunk boundaries, where ACT is otherwise idle and the DVE
                is busy with the normalization chain)."""
                s = (SQ // 128) * c + st
                ps = scpool.tile([128, 1024], f32, tag="sc", name="ps_o")
                for n in range(2):
                    for g in range(2):
                        nc.tensor.matmul(
                            ps[:, n * 512:(n + 1) * 512],
                            lhsT=ag[g][:, s * 128:(s + 1) * 128],
                            rhs=wo[:, g, n * 512:(n + 1) * 512],
                            start=(g == 0), stop=(g == 1),
                        )
                ob = opool.tile([128, 1024], f16, tag="ob", name="ob")
                if ob_act:
                    nc.scalar.copy(ob, ps)
                else:
                    nc.vector.tensor_copy(ob, ps)
                nc.sync.dma_start(out_d[s * 128:(s + 1) * 128, :], ob)

            # ---------------- phase A: K, V, Q(first half) ----------------
            qTop = bpool.tile([128, S], f16)
            qBot = bpool.tile([128, S], f16)
            kTop = bpool.tile([128, S], f16)
            kBot = bpool.tile([128, S], f16)
            kraw = [bpool.tile([128, S], f16, tag=f"kraw{i}", name=f"kraw{i}") for i in range(2)]
            qraw = [bpool.tile([128, S], f16, tag=f"qraw{i}", name=f"qraw{i}") for i in range(2)]
            qR = [bpool.tile([128, S], f16, tag=f"qR{g}", name=f"qR{g}") for g in range(2)]
            kR = [bpool.tile([128, S], f16, tag=f"kR{g}", name=f"kR{g}") for g in range(2)]
            dst_g = {}
            for g in range(2):
                dst_g[id(qR[g])] = g
                dst_g[id(kR[g])] = g

            # K projection + rope + assembly (both halves -- attention needs all
            # kpos), then Q first half so its rope/assembly overlaps V proj.
            for n2 in range(2):
                proj_qk_unit(kraw[0], 2, n2)
                proj_qk_unit(kraw[1], 3, n2)
                rope_unit(kraw[0], kraw[1], kTop, kBot, n2)
                for g in range(2):
                    assemble_unit(kTop, kBot, kR[g], n2)

            # Q projection, first half (chunks 0/1) + rope + assembly
            proj_qk_unit(qraw[0], 0, 0)
            proj_qk_unit(qraw[1], 1, 0)
            rope_unit(qraw[0], qraw[1], qTop, qBot, 0)
            for g in range(2):
                assemble_unit(qTop, qBot, qR[g], 0)

            # V projection into [v_h | 1] stationary tiles
            vOnes = []
            for s in range(ST):
                vo = bpool.tile([128, HC * 65], f16, tag=f"vo{s}", name=f"vo{s}")
                vOnes.append(vo)
                nc.gpsimd.memset(vo, 1.0)
                ps = scpool.tile([128, 1024], f32, tag="sc", name="ps_v")
                for k in range(KT):
                    nc.tensor.matmul(
                        ps[:, :256],
                        lhsT=xT[:, k, s * 128:(s + 1) * 128],
                        rhs=wv[:, k, :],
                        start=(k == 0), stop=(k == KT - 1),
                    )
                nc.vector.tensor_copy(
                    vo.rearrange("p (h w) -> p h w", w=65)[:, :, 0:64],
                    ps[:, :256].rearrange("p (h w) -> p h w", w=64),
                )

            gP = [bpool.tile([128, S], f16, tag=f"gP{g}", name=f"gP{g}") for g in range(2)]
            ag = [bpool.tile([128, S], f16, tag=f"ag{g}", name=f"ag{g}") for g in range(2)]

            # Gate projection + second q half run HERE, in phase A: the Scalar
            # engine is idle (their tanh is free) and, critically, they do NOT
            # occupy score-ring slots during the attention phase -- each such
            # unit used to poke a ~2 us bubble into the exp stream, the true
            # pacer of the attention chunks.
            gate_unit(0, 0)
            gate_unit(1, 0)
            gate_unit(0, 1)
            proj_qk_unit(qraw[0], 0, 1)
            proj_qk_unit(qraw[1], 1, 1)
            rope_unit(qraw[0], qraw[1], qTop, qBot, 1)
            assemble_unit(qTop, qBot, qR[0], 1)
            assemble_unit(qTop, qBot, qR[1], 1)

            def half_unit(fn, *args):
                return [lambda h=h: fn(*args, halves=(h,)) for h in range(2)]

            # Attention-phase fillers: only output-projection units (no ACT
            # cost, short ring occupancy) plus the last gate unit as chunk-0's
            # boundary cover. The LAST TWO of each list run at the chunk
            # boundary, PSUM-unloading through the idle Scalar engine (ob_act).
            fillers = {
                0: [*half_unit(gate_unit, 1, 1)],
                1: [lambda: outproj_unit(0, 0), lambda: outproj_unit(0, 1),
                    lambda: outproj_unit(0, 2, True), lambda: outproj_unit(0, 3, True)],
                2: [lambda: outproj_unit(1, 0), lambda: outproj_unit(1, 1),
                    lambda: outproj_unit(1, 2, True), lambda: outproj_unit(1, 3, True)],
                3: [lambda: outproj_unit(2, 0), lambda: outproj_unit(2, 1),
                    lambda: outproj_unit(2, 2, True), lambda: outproj_unit(2, 3, True)],
            }
            norm_extra = {}

            # ---------------- attention chunks ----------------
            # PV matmuls lag the score/exp pipeline by one sk: they read ex
            # from the 6-deep SBUF ring, so the PE never sits directly behind
            # the Scalar engine in program order and both stream freely.
            scale = float(D) ** -0.5
            pending_norm = None
            carry = []   # boundary fillers, emitted in the NEXT chunk's sk 0/1
            for c in range(NCH):
                csl = slice(c * SQ, (c + 1) * SQ)
                # all 4 heads' PV accumulators in one 4-bank tile; row 64 of
                # bank h collects the softmax denominator via the ones column
                pv4 = pvpool.tile([128, HC, SQ], f32, tag="pv4", name="pv4")
                # memset early (off the boundary critical path): unwritten rows
                # must not hold NaN bit patterns (0 * NaN = NaN in the one-hot
                # broadcast matmul below)
                dstack = wpool.tile([128, SQ], f32, tag="dstack", name="dstack", bufs=2)
                nc.vector.memset(dstack, 1.0)

                def emit_pv(exs, sk):
                    for g in range(2):
                        for h2 in range(2):
                            h = 2 * g + h2
                            nc.tensor.matmul(
                                pv4[0:65, h, :],
                                lhsT=vOnes[sk][:, h * 65:(h + 1) * 65],
                                rhs=exs[g][:, h2 * 512:(h2 + 1) * 512],
                                start=(sk == 0), stop=(sk == ST - 1),
                            )

                todo = list(fillers[c])
                pend = []
                for sk in range(ST):
                    # PV lags TWO sk and is emitted BEFORE the scores: its ex
                    # semaphores are long posted, so it runs immediately and
                    # hides the score-ring slot-grant latency (keeping the PE
                    # dense and its DVFS ramp alive)
                    if len(pend) == 2:
                        emit_pv(*pend.pop(0))
                    exs = []
                    for g in range(2):
                        sct = scpool.tile([128, 1024], f32, tag="sc", name="sct")
                        for h2 in range(2):
                            nc.tensor.matmul(
                                sct[:, h2 * 512:(h2 + 1) * 512],
                                lhsT=kR[g][64 * h2:64 * (h2 + 1), sk * 128:(sk + 1) * 128],
                                rhs=qR[g][64 * h2:64 * (h2 + 1), csl],
                                start=True, stop=True,
                            )
                        ex = expool.tile([128, 1024], f16, tag="ex", name="ex")
                        nc.scalar.activation(ex, sct, Act.Exp, scale=scale)
                        exs.append(ex)
                    # previous chunk's boundary fillers: emitted between this
                    # chunk's first scores and its first PV, so the scores/exp
                    # pipeline restarts instantly at the boundary while the
                    # fillers cover the pv4-release wait
                    if sk < 2 and carry:
                        carry.pop(0)()
                    pend.append((exs, sk))
                    # deferred norm tail of the previous chunk: at sk 3 its
                    # recB tiles slot into the score ring without blocking the
                    # next scores (the reciprocal has drained by then)
                    if sk == 3 and pending_norm is not None:
                        pending_norm()
                        pending_norm = None
                    # spread fillers through the chunk, keeping two for the
                    # chunk boundary; start at sk 5, after the deferred norm
                    # tail (which writes ag) has landed
                    if sk % 2 == 1 and sk >= 5 and len(todo) > 2:
                        todo.pop(0)()
                for p in pend:
                    emit_pv(*p)

                if c == NCH - 1:
                    carry = todo
                    break  # handled by the pipelined tail below

                # ---- normalization, part 1 (DVE, drains under the boundary
                # fillers): denominators stacked on rows {0,32,64,96} so the
                # reciprocal runs partition-parallel (DVE recip costs ~6.3 ns
                # per FREE element), and the gate multiply unloads pv4.
                for h in range(HC):
                    nc.vector.tensor_copy(dstack[32 * h:32 * h + 1, :], pv4[64:65, h, :])
                uus = []
                for h in range(HC):
                    g, h2 = divmod(h, 2)
                    o = 64 * h2
                    uu = wpool.tile([128, SQ], f16, tag=f"uu{h}", name=f"uu{h}", bufs=1)
                    nc.vector.tensor_tensor(uu[o:o + 64, :], pv4[0:64, h, :],
                                            gP[g][o:o + 64, csl], MUL)
                    uus.append(uu)
                rec128 = wpool.tile([128, SQ], f16, tag="rec128", name="rec128", bufs=2)
                with nc.allow_low_precision(reason="fp16 denom recip: denom in [2.4e3, 9e3], rel err 2^-11"):
                    nc.vector.reciprocal(out=rec128, in_=dstack)

                # leftover fillers become the next chunk's boundary cover
                carry = todo

                # ---- normalization, part 2 (deferred into the next chunk's
                # sk loop): one-hot fp16 matmuls broadcast each head's
                # reciprocal row across partitions, then ag = uu * recB.
                def make_norm_tail(c=c, csl=csl, rec128=rec128, uus=uus):
                    def fin():
                        for g in range(2):
                            recB2 = scpool.tile([128, 1024], f32, tag="sc", name="recB2")
                            for h2 in range(2):
                                h = 2 * g + h2
                                nc.tensor.matmul(recB2[:, h2 * 512:(h2 + 1) * 512],
                                                 lhsT=hot[:, 128 * h:128 * (h + 1)],
                                                 rhs=rec128, start=True, stop=True)
                            for h2 in range(2):
                                h = 2 * g + h2
                                o = 64 * h2
                                nc.vector.tensor_tensor(
                                    ag[g][o:o + 64, csl],
                                    uus[h][o:o + 64, :],
                                    recB2[o:o + 64, h2 * 512:(h2 + 1) * 512],
                                    MUL,
                                )
                        extra = norm_extra.get(c)
                        if extra:
                            extra()
                    return fin
                pending_norm = make_norm_tail()

            # ---- pipelined tail: the last chunk's normalization runs in two
            # 256-column halves so the output projection of half A overlaps
            # half B's reciprocal chain; all PSUM unloads go through ACT.
            c3 = NCH - 1
            csl3 = slice(c3 * SQ, (c3 + 1) * SQ)
            rec128 = wpool.tile([128, SQ], f16, tag="rec128", name="rec128", bufs=2)
            uus = [wpool.tile([128, SQ], f16, tag=f"uu{h}", name=f"uu{h}", bufs=1)
                   for h in range(HC)]
            first = True
            for qo in (0, 256):
                qsl = slice(qo, qo + 256)
                for h in range(HC):
                    nc.vector.tensor_copy(dstack[32 * h:32 * h + 1, qsl],
                                          pv4[64:65, h, qsl])
                for h in range(HC):
                    g, h2 = divmod(h, 2)
                    o = 64 * h2
                    nc.vector.tensor_tensor(
                        uus[h][o:o + 64, qsl], pv4[0:64, h, qsl],
                        gP[g][o:o + 64, c3 * SQ + qo:c3 * SQ + qo + 256], MUL)
                with nc.allow_low_precision(reason="fp16 denom recip"):
                    nc.vector.reciprocal(out=rec128[:, qsl], in_=dstack[:, qsl])
                if first:
                    for f in carry:  # PE cover for half A's chain
                        f()
                    first = False
                for g in range(2):
                    recB2 = scpool.tile([128, 1024], f32, tag="sc", name="recB2")
                    for h2 in range(2):
                        h = 2 * g + h2
                        nc.tensor.matmul(recB2[:, h2 * 256:(h2 + 1) * 256],
                                         lhsT=hot[:, 128 * h:128 * (h + 1)],
                                         rhs=rec128[:, qsl], start=True, stop=True)
                    for h2 in range(2):
                        h = 2 * g + h2
                        o = 64 * h2
                        nc.vector.tensor_tensor(
                            ag[g][o:o + 64, c3 * SQ + qo:c3 * SQ + qo + 256],
                            uus[h][o:o + 64, qsl],
                            recB2[o:o + 64, h2 * 256:(h2 + 1) * 256],
                            MUL,
                        )
                for st in (0, 1) if qo == 0 else (2, 3):
                    outproj_unit(c3, st, ob_act=True)

    return nc


def _host_inputs(x, w_qkv, w_gate, w_out):
    """Build the 8 per-core input maps (all device tensors fp16)."""
    f16 = np.float16
    x = np.asarray(x, dtype=np.float32)
    w_qkv = np.asarray(w_qkv, dtype=np.float32)
    w_gate = np.asarray(w_gate, dtype=np.float32)
    w_out = np.asarray(w_out, dtype=np.float32)

    inv = 1.0 / (ROPE_THETA ** (np.arange(0, D, 2, dtype=np.float64) / D))   # [32]
    ang = np.arange(S, dtype=np.float64)[None, :] * inv[:, None]             # [32, S]
    cs = np.tile(np.cos(ang), (4, 1)).astype(f16)                            # [128, S]
    sn = np.tile(np.sin(ang), (4, 1)).astype(f16)

    wq = w_qkv[:, 0:E]
    wk = w_qkv[:, E:2 * E]
    wvv = w_qkv[:, 2 * E:3 * E]

    in_maps = []
    for c in range(NCORES):
        b = c // 4
        hs = HC * (c % 4)
        cols_ev = np.concatenate([(hs + h) * 64 + np.arange(0, 64, 2) for h in range(HC)])
        cols_od = cols_ev + 1
        wqk_p = np.concatenate(
            [wq[:, cols_ev], wq[:, cols_od], wk[:, cols_ev], wk[:, cols_od]], axis=1)
        vcols = np.concatenate([(hs + h) * 64 + np.arange(64) for h in range(HC)])
        wo_p = w_out[vcols, :].reshape(2, 128, E).transpose(1, 0, 2)
        in_maps.append({
            "xT": np.ascontiguousarray(x[b].T).astype(f16),
            "wqk": np.ascontiguousarray(wqk_p).astype(f16),
            "wv": np.ascontiguousarray(wvv[:, vcols]).astype(f16),
            "wg": np.ascontiguousarray(w_gate[:, vcols]).astype(f16),
            "wo": np.ascontiguousarray(wo_p).astype(f16),
            "cs": cs,
            "sn": sn,
        })
    return in_maps


def kernel(x, w_qkv, w_gate, w_out, b_out, n_heads):
    global LAST_RESULTS
    assert int(n_heads) == H
    x = np.asarray(x)
    assert x.shape == (B, S, E)

    from concourse.bass_utils import run_bass_kernel_spmd

    _install_birfix()
    if "nc" not in _CACHE:
        _CACHE["nc"] = _build_nc()
    nc = _CACHE["nc"]

    in_maps = _host_inputs(x, w_qkv, w_gate, w_out)
    import os
    trace = bool(int(os.environ.get("KERNEL_TRACE", "0")))
    tmpdir = os.environ.get("KERNEL_TRACE_DIR") if trace else None
    res = run_bass_kernel_spmd(nc, in_maps, list(range(NCORES)), trace=trace,
                               tmpdir=tmpdir)
    LAST_RESULTS = res

    out = np.zeros((B, S, E), dtype=np.float32)
    for c in range(NCORES):
        out[c // 4] += res.results[c]["out"].astype(np.float32)
    out += np.asarray(b_out, dtype=np.float32)[None, None, :]
    return out
